# revision 1
# baseline (speedup 1.0000x reference)
"""Trainium2 Bass kernel for the 10-class supervised-contrastive loss.

Problem shapes (hardcoded): preds [10, 2048, 128] f32, target [2048] int64,
log_vars [10] f32 -> scalar f32.

Sharding (8 cores, SPMD, identical program per core):
  - core c owns class c fully (16 row-strips of 128 rows of the [B,B] matrix)
  - cores 0-3 additionally own a quarter of class 8, cores 4-7 a quarter of
    class 9.  The extra class's rows/labels are fed ROTATED (np.roll) so every
    core statically computes row-strips 0..3 of its "slot 1" class; row sums /
    masked sums are permutation-invariant so rotation is safe, and the
    diagonal stays on the diagonal.

Host prep (O(B*D) layout/scaling only): row-normalize features, cast bf16,
provide both layouts (G = [d, b] transposed, gh = b-tiled [b%128, t*128+d]),
one-hot labels.  ALL O(B^2) work runs on device.

Device, per class:
  per 128-row strip a (slot 0 computes only the upper trapezoid cols >= a*128,
  exploiting symmetry of exp(cos/T)):
      C = G[:,a].T @ G[:, cols]     (bf16 matmuls, f32 PSUM, 512-col chunks)
      zero diagonal window          (DVE mul with (1-I))
      E = Exp(C/T) -> sc (bf16)     (ACT, fused accum_out row-sum)
      col-sums of computed tiles    (PE matmuls E_tile.T @ ones, accumulated
                                     per target strip in one PSUM bank) give
                                     the row-sums of the skipped lower tiles.
  U = gh.T @ [onehot, ones] ; P_a = G_a . u_pos, R_a = G_a . u_all  (PE)
  outputs per row: [rowsum(E), P, R] -> out [128, 60]

Host epilogue (O(B*C)): Z = rowsum - 1 (diag contributed exp(0)=1), masked
mean log-prob from P/R + analytic counts, uncertainty-weighted final sum.
"""

import ml_dtypes
import numpy as np

import concourse.bacc as bacc
import concourse.bass as bass
import concourse.mybir as mybir
import concourse.tile as tile
from concourse.bass_utils import run_bass_kernel_spmd

NUM_CLASSES = 10
B = 2048
D = 128
T = 0.07
BASE_T = 0.07
N_CORES = 8

f32 = mybir.dt.float32
bf16 = mybir.dt.bfloat16
np_bf16 = ml_dtypes.bfloat16

# (slot, row_strip) units every core executes, in order.
UNITS = [(0, rb) for rb in range(16)] + [(1, rb) for rb in range(4)]

TRACE = False
LAST_RESULT = None


def _chunks(c0, c1):
    """Split [c0, c1) at 512-aligned boundaries (PSUM bank limit)."""
    out = []
    c = c0
    while c < c1:
        nxt = min(c1, (c // 512 + 1) * 512)
        out.append((c, nxt))
        c = nxt
    return out


def _build_nc():
    nc = bacc.Bacc(None, target_bir_lowering=False)

    g_dram = [
        nc.dram_tensor(f"g{s}", [128, B], bf16, kind="ExternalInput")
        for s in range(2)
    ]
    gh_dram = [
        nc.dram_tensor(f"gh{s}", [128, B], bf16, kind="ExternalInput")
        for s in range(2)
    ]
    lw_dram = [
        nc.dram_tensor(f"lw{s}", [128, 32], bf16, kind="ExternalInput")
        for s in range(2)
    ]
    masknd_dram = nc.dram_tensor("masknd", [128, 128], f32, kind="ExternalInput")
    onesf_dram = nc.dram_tensor("onesf", [128, 1], f32, kind="ExternalInput")
    out_dram = nc.dram_tensor("out", [128, 3 * len(UNITS)], f32, kind="ExternalOutput")

    add = mybir.AluOpType.add
    EXP = mybir.ActivationFunctionType.Exp

    with tile.TileContext(nc) as tc:
        with (
            tc.tile_pool(name="const", bufs=1) as constp,
            tc.tile_pool(name="gmat", bufs=1) as gmatp,
            tc.tile_pool(name="scp", bufs=4) as scp,
        ):
            masknd_sb = constp.tile([128, 128], f32, tag="masknd")
            nc.sync.dma_start(masknd_sb[:], masknd_dram[:])
            onesf_sb = constp.tile([128, 1], f32, tag="onesf")
            nc.sync.dma_start(onesf_sb[:], onesf_dram[:])
            out_sb = constp.tile([128, 3 * len(UNITS) + 2], f32, tag="out")

            G = []
            GH = []
            LW = []
            for s in range(2):
                g = gmatp.tile([128, B], bf16, tag=f"G{s}", name=f"G{s}")
                nc.sync.dma_start(g[:], g_dram[s][:])
                G.append(g)
                gh = gmatp.tile([128, B], bf16, tag=f"GH{s}", name=f"GH{s}")
                nc.sync.dma_start(gh[:], gh_dram[s][:])
                GH.append(gh)
                lw = gmatp.tile([128, 32], bf16, tag=f"LW{s}", name=f"LW{s}")
                nc.sync.dma_start(lw[:], lw_dram[s][:])
                LW.append(lw)
            Usb = [
                gmatp.tile([128, 2], bf16, tag=f"U{s}", name=f"U{s}") for s in range(2)
            ]

            with (
                tc.tile_pool(name="upsum", bufs=2, space="PSUM") as upp,
                tc.tile_pool(name="prpsum", bufs=2, space="PSUM") as prp,
            ):
                for s in range(2):
                    # U = sum_b gh[b,:]^T * [onehot_b, 1]  (accumulate)
                    up = upp.tile([128, 2], f32, tag="up")
                    for t in range(16):
                        nc.tensor.matmul(
                            up[:],
                            GH[s][:, bass.ts(t, 128)],
                            LW[s][:, bass.ts(t, 2)],
                            start=(t == 0),
                            stop=(t == 15),
                        )
                    nc.vector.tensor_copy(Usb[s][:], up[:])

                # P_a = G_a . u_pos, R_a = G_a . u_all for every row strip.
                for u, (s, rb) in enumerate(UNITS):
                    pr = prp.tile([128, 2], f32, tag="pr")
                    nc.tensor.matmul(
                        pr[:],
                        G[s][:, bass.ts(rb, 128)],
                        Usb[s][:],
                        start=True,
                        stop=True,
                    )
                    nc.vector.tensor_copy(out_sb[:, 3 * u + 1 : 3 * u + 3], pr[:])

            with tc.tile_pool(name="mainpsum", bufs=3, space="PSUM") as cpp:
                # Column-sum accumulator: col rb = sum of E-tile column sums
                # from strips < rb (the transposed/skipped lower tiles).
                csum = cpp.tile([128, 16], f32, tag="csum", bufs=1, name="csum")
                ones_col = onesf_sb[:]  # [128,1] f32 ones

                for u, (s, rb) in enumerate(UNITS):
                    lhsT = G[s][:, bass.ts(rb, 128)]
                    if s == 0:
                        halves = (
                            [(0, rb * 128, 1024), (1024, 1024, 2048)]
                            if rb < 8
                            else [(1024, rb * 128, 2048)]
                        )
                    else:
                        halves = [(0, 0, 1024), (1024, 1024, 2048)]
                    acc_cols = []
                    for hi, (base, c0, c1) in enumerate(halves):
                        cp = cpp.tile([128, 1024], f32, tag="cp", name=f"cp{u}_{hi}")
                        for a0, a1 in _chunks(c0, c1):
                            nc.tensor.matmul(
                                cp[:, a0 - base : a1 - base],
                                lhsT,
                                G[s][:, a0:a1],
                                start=True,
                                stop=True,
                            )
                        if c0 <= rb * 128 < c1:
                            # Zero the diagonal window.
                            w0 = rb * 128 - base
                            nc.vector.tensor_mul(
                                cp[:, w0 : w0 + 128],
                                cp[:, w0 : w0 + 128],
                                masknd_sb[:],
                            )
                        sc = scp.tile([128, 1024], f32, tag="sc", name=f"sc{u}_{hi}")
                        if hi == 0:
                            acol = out_sb[:, 3 * u : 3 * u + 1]
                        else:
                            acol = out_sb[:, 60 + (u % 2) : 61 + (u % 2)]
                        acc_cols.append(acol)
                        nc.scalar.activation(
                            sc[:, c0 - base : c1 - base],
                            cp[:, c0 - base : c1 - base],
                            EXP,
                            scale=1.0 / T,
                            accum_out=acol,
                        )
                        if s == 0:
                            # Column sums of computed tiles feed the row sums
                            # of the mirrored (skipped) tiles.
                            for cb in range(max(rb + 1, c0 // 128), c1 // 128):
                                nc.tensor.matmul(
                                    csum[:, cb : cb + 1],
                                    sc[:, cb * 128 - base : cb * 128 - base + 128],
                                    ones_col,
                                    start=(rb == 0),
                                    stop=(rb == cb - 1),
                                    skip_group_check=True,
                                )
                    if len(acc_cols) == 2:
                        nc.vector.tensor_tensor(
                            out=acc_cols[0], in0=acc_cols[0], in1=acc_cols[1], op=add
                        )
                    if s == 0 and rb > 0:
                        nc.vector.tensor_tensor(
                            out=out_sb[:, 3 * u : 3 * u + 1],
                            in0=out_sb[:, 3 * u : 3 * u + 1],
                            in1=csum[:, rb : rb + 1],
                            op=add,
                        )

            nc.sync.dma_start(out_dram[:], out_sb[:, 0 : 3 * len(UNITS)])
    nc.finalize()
    return nc


_NC_CACHE = None


def _get_nc():
    global _NC_CACHE
    if _NC_CACHE is None:
        _NC_CACHE = _build_nc()
    return _NC_CACHE


def kernel(preds, target, log_vars):
    global LAST_RESULT
    preds = np.asarray(preds, dtype=np.float32)
    target = np.asarray(target)
    log_vars = np.asarray(log_vars, dtype=np.float32)

    onehot = (target[None, :] == np.arange(NUM_CLASSES, dtype=target.dtype)[:, None])
    onehot = onehot.astype(np.float32)  # [10, B]
    npos = onehot.sum(axis=1).astype(np.float64)  # [10]

    # Host prep: row-normalize (f32 stats), cast bf16, build both layouts.
    norms = np.sqrt((preds.astype(np.float32) ** 2).sum(axis=2, dtype=np.float32))
    ghat = (preds / norms[:, :, None]).astype(np_bf16)  # [10, B, D]

    masknd = np.ascontiguousarray(1.0 - np.eye(128, dtype=np.float32))

    in_maps = []
    for c in range(N_CORES):
        cls1 = 8 + c // 4
        off = 512 * (c % 4)
        im = {"masknd": masknd, "onesf": np.ones((128, 1), np.float32)}
        for s, (cls, o) in enumerate([(c, 0), (cls1, off)]):
            gh = np.roll(ghat[cls], -o, axis=0) if o else ghat[cls]
            lab = np.roll(onehot[cls], -o) if o else onehot[cls]
            im[f"g{s}"] = np.ascontiguousarray(gh.T)  # [128, 2048] [d, b]
            im[f"gh{s}"] = np.ascontiguousarray(
                gh.reshape(16, 128, 128).transpose(1, 0, 2).reshape(128, 2048)
            )  # [b%128, t*128+d]
            lw = np.ones((128, 16, 2), dtype=np_bf16)
            lw[:, :, 0] = lab.reshape(16, 128).T
            im[f"lw{s}"] = np.ascontiguousarray(lw.reshape(128, 32))
        in_maps.append(im)

    nc = _get_nc()
    res = run_bass_kernel_spmd(nc, in_maps, list(range(N_CORES)), trace=TRACE)
    LAST_RESULT = res

    # Reassemble per-(class,row) stats.
    zpr = np.zeros((NUM_CLASSES, B, 3), dtype=np.float64)
    rows128 = np.arange(128)
    for c in range(N_CORES):
        o = np.asarray(res.results[c]["out"], dtype=np.float64)  # [128, 60]
        for u, (s, rb) in enumerate(UNITS):
            if s == 0:
                cls, base = c, 0
            else:
                cls, base = 8 + c // 4, 512 * (c % 4)
            rows = (base + rb * 128 + rows128) % B
            zpr[cls, rows, :] = o[:, 3 * u : 3 * u + 3]

    Z = zpr[:, :, 0] - 1.0  # remove diag exp(0)=1 contribution
    P = zpr[:, :, 1]
    R = zpr[:, :, 2]
    lab = onehot.astype(np.float64)
    masked_cos = lab * P + (1.0 - lab) * (R - P)
    masked_logits_sum = (masked_cos - 1.0) / T
    cnt = lab * npos[:, None] + (1.0 - lab) * (B - npos[:, None]) - 1.0
    mlpp = masked_logits_sum / cnt - np.log(Z)
    losses = -(T / BASE_T) * mlpp.mean(axis=1)  # [10]
    lv = log_vars.astype(np.float64)
    final = np.sum(np.exp(-lv) * losses + lv)
    return np.float32(final)



# revision 6
# speedup vs baseline: 1.6260x; 1.6260x over previous
"""Trainium2 Bass kernel for the 10-class supervised-contrastive loss.

Problem shapes (hardcoded): preds [10, 2048, 128] f32, target [2048] int64,
log_vars [10] f32 -> scalar f32.

Sharding (8 cores, SPMD, identical program per core):
  - core c owns class c fully (16 row-strips of 128 rows of the [B,B] matrix)
  - cores 0-3 additionally own a quarter of class 8, cores 4-7 a quarter of
    class 9.  The extra class's rows/labels are fed ROTATED (np.roll) so every
    core statically computes row-strips 0..3 of its "slot 1" class; row sums /
    masked sums are permutation-invariant so rotation is safe, and the
    diagonal stays on the diagonal.

Host prep (O(B*D) layout/scaling only): row-normalize features, cast bf16,
provide both layouts (G = [d, b] transposed, gh = b-tiled [b%128, t*128+d]),
one-hot labels.  ALL O(B^2) work runs on device.

Device, per class:
  per 128-row strip a (slot 0 computes only the upper trapezoid cols >= a*128,
  exploiting symmetry of exp(cos/T)):
      C = G[:,a].T @ G[:, cols]     (bf16 matmuls, f32 PSUM, 512-col chunks)
      zero diagonal window          (DVE mul with (1-I))
      E = Exp(C/T) -> sc (bf16)     (ACT, fused accum_out row-sum)
      col-sums of computed tiles    (PE matmuls E_tile.T @ ones, accumulated
                                     per target strip in one PSUM bank) give
                                     the row-sums of the skipped lower tiles.
  U = gh.T @ [onehot, ones] ; P_a = G_a . u_pos, R_a = G_a . u_all  (PE)
  outputs per row: [rowsum(E), P, R] -> out [128, 60]

Host epilogue (O(B*C)): Z = rowsum - 1 (diag contributed exp(0)=1), masked
mean log-prob from P/R + analytic counts, uncertainty-weighted final sum.
"""

import ml_dtypes
import numpy as np

import concourse.bacc as bacc
import concourse.bass as bass
import concourse.mybir as mybir
import concourse.tile as tile
from concourse.bass_utils import run_bass_kernel_spmd

NUM_CLASSES = 10
B = 2048
D = 128
T = 0.07
BASE_T = 0.07
N_CORES = 8

f32 = mybir.dt.float32
bf16 = mybir.dt.bfloat16
np_bf16 = ml_dtypes.bfloat16

# (slot, row_strip) units every core executes, in order.
UNITS = [(0, rb) for rb in range(16)] + [(1, rb) for rb in range(4)]

TRACE = False
LAST_RESULT = None


def _chunks(c0, c1):
    """Split [c0, c1) at 512-aligned boundaries (PSUM bank limit)."""
    out = []
    c = c0
    while c < c1:
        nxt = min(c1, (c // 512 + 1) * 512)
        out.append((c, nxt))
        c = nxt
    return out


def _build_nc():
    nc = bacc.Bacc(None, target_bir_lowering=False)

    g_dram = [
        nc.dram_tensor(f"g{s}", [128, B], bf16, kind="ExternalInput")
        for s in range(2)
    ]
    gh_dram = [
        nc.dram_tensor(f"gh{s}", [128, B], bf16, kind="ExternalInput")
        for s in range(2)
    ]
    lw_dram = [
        nc.dram_tensor(f"lw{s}", [128, 32], bf16, kind="ExternalInput")
        for s in range(2)
    ]
    masknd_dram = nc.dram_tensor("masknd", [128, 128], f32, kind="ExternalInput")
    onesf_dram = nc.dram_tensor("onesf", [128, 1], bf16, kind="ExternalInput")
    out_dram = nc.dram_tensor("out", [128, 3 * len(UNITS)], f32, kind="ExternalOutput")

    add = mybir.AluOpType.add
    EXP = mybir.ActivationFunctionType.Exp

    with tile.TileContext(nc) as tc:
        with (
            tc.tile_pool(name="const", bufs=1) as constp,
            tc.tile_pool(name="gmat", bufs=1) as gmatp,
            tc.tile_pool(name="scp", bufs=4) as scp,
        ):
            masknd_sb = constp.tile([128, 128], f32, tag="masknd")
            nc.sync.dma_start(masknd_sb[:], masknd_dram[:])
            onesf_sb = constp.tile([128, 1], bf16, tag="onesf")
            nc.sync.dma_start(onesf_sb[:], onesf_dram[:])
            out_sb = constp.tile([128, 3 * len(UNITS) + 2], f32, tag="out")

            G = []
            GH = []
            LW = []
            for s in range(2):
                g = gmatp.tile([128, B], bf16, tag=f"G{s}", name=f"G{s}")
                nc.sync.dma_start(g[:], g_dram[s][:])
                G.append(g)
                gh = gmatp.tile([128, B], bf16, tag=f"GH{s}", name=f"GH{s}")
                nc.sync.dma_start(gh[:], gh_dram[s][:])
                GH.append(gh)
                lw = gmatp.tile([128, 32], bf16, tag=f"LW{s}", name=f"LW{s}")
                nc.sync.dma_start(lw[:], lw_dram[s][:])
                LW.append(lw)
            Usb = [
                gmatp.tile([128, 2], bf16, tag=f"U{s}", name=f"U{s}") for s in range(2)
            ]

            with (
                tc.tile_pool(name="upsum", bufs=2, space="PSUM") as upp,
                tc.tile_pool(name="prpsum", bufs=2, space="PSUM") as prp,
            ):
                for s in range(2):
                    # U = sum_b gh[b,:]^T * [onehot_b, 1]  (accumulate)
                    up = upp.tile([128, 2], f32, tag="up")
                    for t in range(16):
                        nc.tensor.matmul(
                            up[:],
                            GH[s][:, bass.ts(t, 128)],
                            LW[s][:, bass.ts(t, 2)],
                            start=(t == 0),
                            stop=(t == 15),
                        )
                    nc.vector.tensor_copy(Usb[s][:], up[:])

                # P_a = G_a . u_pos, R_a = G_a . u_all for every row strip.
                for u, (s, rb) in enumerate(UNITS):
                    pr = prp.tile([128, 2], f32, tag="pr")
                    nc.tensor.matmul(
                        pr[:],
                        G[s][:, bass.ts(rb, 128)],
                        Usb[s][:],
                        start=True,
                        stop=True,
                    )
                    nc.vector.tensor_copy(out_sb[:, 3 * u + 1 : 3 * u + 3], pr[:])

            with tc.tile_pool(name="mainpsum", bufs=3, space="PSUM") as cpp:
                # Column-sum accumulator: col rb = sum of E-tile column sums
                # from strips < rb (the transposed/skipped lower tiles).
                csum = cpp.tile([128, 16], f32, tag="csum", bufs=1, name="csum")
                ones_col = onesf_sb[:]  # [128,1] bf16 ones

                for u, (s, rb) in enumerate(UNITS):
                    lhsT = G[s][:, bass.ts(rb, 128)]
                    if s == 0:
                        halves = (
                            [(0, rb * 128, 1024), (1024, 1024, 2048)]
                            if rb < 8
                            else [(1024, rb * 128, 2048)]
                        )
                    else:
                        halves = [(0, 0, 1024), (1024, 1024, 2048)]
                    acc_cols = []
                    for hi, (base, c0, c1) in enumerate(halves):
                        cp = cpp.tile([128, 1024], f32, tag="cp", name=f"cp{u}_{hi}")
                        for a0, a1 in _chunks(c0, c1):
                            nc.tensor.matmul(
                                cp[:, a0 - base : a1 - base],
                                lhsT,
                                G[s][:, a0:a1],
                                start=True,
                                stop=True,
                            )
                        if c0 <= rb * 128 < c1:
                            # Zero the diagonal window.
                            w0 = rb * 128 - base
                            nc.vector.tensor_mul(
                                cp[:, w0 : w0 + 128],
                                cp[:, w0 : w0 + 128],
                                masknd_sb[:],
                            )
                        sc = scp.tile([128, 1024], bf16, tag="sc", name=f"sc{u}_{hi}")
                        if hi == 0:
                            acol = out_sb[:, 3 * u : 3 * u + 1]
                        else:
                            acol = out_sb[:, 60 + (u % 2) : 61 + (u % 2)]
                        acc_cols.append(acol)
                        nc.scalar.activation(
                            sc[:, c0 - base : c1 - base],
                            cp[:, c0 - base : c1 - base],
                            EXP,
                            scale=1.0 / T,
                            accum_out=acol,
                        )
                        if s == 0:
                            # Column sums of computed tiles feed the row sums
                            # of the mirrored (skipped) tiles.
                            for cb in range(max(rb + 1, c0 // 128), c1 // 128):
                                nc.tensor.matmul(
                                    csum[:, cb : cb + 1],
                                    sc[:, cb * 128 - base : cb * 128 - base + 128],
                                    ones_col,
                                    start=(rb == 0),
                                    stop=(rb == cb - 1),
                                    skip_group_check=True,
                                )
                    if len(acc_cols) == 2:
                        nc.vector.tensor_tensor(
                            out=acc_cols[0], in0=acc_cols[0], in1=acc_cols[1], op=add
                        )
                    if s == 0 and rb > 0:
                        nc.vector.tensor_tensor(
                            out=out_sb[:, 3 * u : 3 * u + 1],
                            in0=out_sb[:, 3 * u : 3 * u + 1],
                            in1=csum[:, rb : rb + 1],
                            op=add,
                        )

            nc.sync.dma_start(out_dram[:], out_sb[:, 0 : 3 * len(UNITS)])
    nc.finalize()
    return nc


_NC_CACHE = None


def _get_nc():
    global _NC_CACHE
    if _NC_CACHE is None:
        _NC_CACHE = _build_nc()
    return _NC_CACHE


def kernel(preds, target, log_vars):
    global LAST_RESULT
    preds = np.asarray(preds, dtype=np.float32)
    target = np.asarray(target)
    log_vars = np.asarray(log_vars, dtype=np.float32)

    onehot = (target[None, :] == np.arange(NUM_CLASSES, dtype=target.dtype)[:, None])
    onehot = onehot.astype(np.float32)  # [10, B]
    npos = onehot.sum(axis=1).astype(np.float64)  # [10]

    # Host prep: row-normalize (f32 stats), cast bf16, build both layouts.
    norms = np.sqrt((preds.astype(np.float32) ** 2).sum(axis=2, dtype=np.float32))
    ghat = (preds / norms[:, :, None]).astype(np_bf16)  # [10, B, D]

    masknd = np.ascontiguousarray(1.0 - np.eye(128, dtype=np.float32))

    in_maps = []
    for c in range(N_CORES):
        cls1 = 8 + c // 4
        off = 512 * (c % 4)
        im = {"masknd": masknd, "onesf": np.ones((128, 1), np_bf16)}
        for s, (cls, o) in enumerate([(c, 0), (cls1, off)]):
            gh = np.roll(ghat[cls], -o, axis=0) if o else ghat[cls]
            lab = np.roll(onehot[cls], -o) if o else onehot[cls]
            im[f"g{s}"] = np.ascontiguousarray(gh.T)  # [128, 2048] [d, b]
            im[f"gh{s}"] = np.ascontiguousarray(
                gh.reshape(16, 128, 128).transpose(1, 0, 2).reshape(128, 2048)
            )  # [b%128, t*128+d]
            lw = np.ones((128, 16, 2), dtype=np_bf16)
            lw[:, :, 0] = lab.reshape(16, 128).T
            im[f"lw{s}"] = np.ascontiguousarray(lw.reshape(128, 32))
        in_maps.append(im)

    nc = _get_nc()
    res = run_bass_kernel_spmd(nc, in_maps, list(range(N_CORES)), trace=TRACE)
    LAST_RESULT = res

    # Reassemble per-(class,row) stats.
    zpr = np.zeros((NUM_CLASSES, B, 3), dtype=np.float64)
    rows128 = np.arange(128)
    for c in range(N_CORES):
        o = np.asarray(res.results[c]["out"], dtype=np.float64)  # [128, 60]
        for u, (s, rb) in enumerate(UNITS):
            if s == 0:
                cls, base = c, 0
            else:
                cls, base = 8 + c // 4, 512 * (c % 4)
            rows = (base + rb * 128 + rows128) % B
            zpr[cls, rows, :] = o[:, 3 * u : 3 * u + 3]

    Z = zpr[:, :, 0] - 1.0  # remove diag exp(0)=1 contribution
    P = zpr[:, :, 1]
    R = zpr[:, :, 2]
    lab = onehot.astype(np.float64)
    masked_cos = lab * P + (1.0 - lab) * (R - P)
    masked_logits_sum = (masked_cos - 1.0) / T
    cnt = lab * npos[:, None] + (1.0 - lab) * (B - npos[:, None]) - 1.0
    mlpp = masked_logits_sum / cnt - np.log(Z)
    losses = -(T / BASE_T) * mlpp.mean(axis=1)  # [10]
    lv = log_vars.astype(np.float64)
    final = np.sum(np.exp(-lv) * losses + lv)
    return np.float32(final)

